# revision 2
# baseline (speedup 1.0000x reference)
"""ConvLSTM Trainium2 kernel (8 NeuronCores, data-parallel over batch).

Math (per timestep t, batched over B):
  att  = softmax(tanh(mean_s(x) @ fc1.T + b1) @ fc2.T + b2)          [B, C]
  y    = conv3d(x * att) + conv_b  -> flatten                         [B, 1728]
  gates= y @ w_ih.T + b_ih + h @ w_hh.T + b_hh                        [B, 256]
  LSTM cell -> h_t; out = mean_t(h_t) @ fc_w.T + fc_b                 [B, 3]

Key restructuring:
  * conv3d on a 3x3x3 grid with same padding is a linear map; fold it into
    the LSTM input projection on host: W_big = w_ih @ W_conv  [256, 1728].
    The whole per-timestep feedforward then batches over all B*T tokens.
  * Only the LSTM recurrence is sequential. All gate nonlinearities are
    expressed via tanh (sigmoid(z) = 0.5 + 0.5*tanh(z/2)); the g-gate rows
    are pre-scaled by 2 on host so ONE activation(tanh, scale=0.5) covers
    all four gates. The cell carries doubled states S=2c, H=2h so the cell
    update is exactly 4 DVE scalar_tensor_tensor ops:
       u1 = (tau_f + 1) * S ; u2 = (tau_i + 1) * tau_g
       S' = 0.5*u1 + u2     ; tanh_c = tanh(0.5*S') [ACT]
       H' = (tau_o + 1) * tanh_c
    (w_hh folded by 0.5 for the H carry; fc_w folded by 0.5/T for the mean.)
  * gates_x[t] is pre-loaded into PSUM with an identity matmul so the two
    small recurrent matmuls accumulate on top of it and the gate tanh reads
    PSUM directly.

Sharding: batch 128 -> 16 per core. Token layout t-major (tok = t*16 + b).
x is transposed host-side to [k=(c,s), tok] so the contraction dim lands on
partitions with fully contiguous DMA.
"""

import numpy as np
from contextlib import ExitStack

import concourse.bass as bass
import concourse.tile as tile
import concourse.mybir as mybir
from concourse import bacc
from concourse.bass_utils import run_bass_kernel_spmd
from concourse.masks import make_identity

FP32 = mybir.dt.float32
AL = mybir.AluOpType

B, T, C = 128, 128, 64
HID = 64
S3 = 27                    # 3*3*3 spatial positions
KIN = C * S3               # 1728
NCH = 14                   # contraction chunks of 128 (padded)
KPAD = NCH * 128           # 1792
NCORES = 8
BL = B // NCORES           # 16 batch per core
NTOK = BL * T              # 2048 tokens per core
NBLK = 4
BLKTOK = NTOK // NBLK      # 512 tokens per block
TBLK = T // NBLK           # 32 timesteps per block
NG = BLKTOK // 128         # 4 transpose quads per block

_CACHE = {}


# ---------------------------------------------------------------- host folds
def _conv_matrix(conv_w):
    """[HID, C, 3, 3, 3] -> dense [HID*27, C*27] linear map of the same-padded
    3x3x3 conv on a 3x3x3 grid."""
    pos = np.arange(S3)
    pz, py, px = pos // 9, (pos // 3) % 3, pos % 3
    rows = np.arange(HID) * S3
    cols = np.arange(C) * S3
    Wc = np.zeros((HID * S3, C * S3), np.float32)
    for p in range(S3):
        for q in range(S3):
            kz = pz[q] - pz[p] + 1
            ky = py[q] - py[p] + 1
            kx = px[q] - px[p] + 1
            if 0 <= kz < 3 and 0 <= ky < 3 and 0 <= kx < 3:
                Wc[np.ix_(rows + p, cols + q)] = conv_w[:, :, kz, ky, kx]
    return Wc


def _fold_weights(fc1_w, fc1_b, fc2_w, fc2_b, conv_w, conv_b,
                  w_ih, w_hh, b_ih, b_hh, fc_w, fc_b):
    Wc = _conv_matrix(np.asarray(conv_w, np.float32))
    w_ih = np.asarray(w_ih, np.float32)
    W_big = (w_ih.astype(np.float64) @ Wc.astype(np.float64)).astype(np.float32)
    b_all = (w_ih @ np.repeat(np.asarray(conv_b, np.float32), S3)
             + np.asarray(b_ih, np.float32) + np.asarray(b_hh, np.float32))
    g = slice(2 * HID, 3 * HID)            # g-gate rows (torch order i,f,g,o)
    W_big = W_big.copy(); b_all = b_all.copy()
    W_big[g] *= 2.0
    b_all[g] *= 2.0
    whh_eff = np.asarray(w_hh, np.float32) * 0.5
    whh_eff = whh_eff.copy(); whh_eff[g] *= 2.0
    fcw_eff = np.asarray(fc_w, np.float32) * (0.5 / T)

    WbT = np.zeros((KPAD, 256), np.float32)
    WbT[:KIN] = W_big.T
    WbT = np.ascontiguousarray(WbT.reshape(NCH, 128, 256))

    k = np.arange(KPAD)
    cid = np.where(k < KIN, k // S3, -1)
    Em = (cid[:, None] == np.arange(C)[None, :]).astype(np.float32) / S3
    Em = np.ascontiguousarray(Em.reshape(NCH, 128, C))               # mean
    Eb = (np.arange(C)[:, None] == cid[None, :]).astype(np.float32)  # bcast
    Eb = np.ascontiguousarray(Eb.reshape(C, NCH, 128).transpose(1, 0, 2))

    return {
        "wbigT": WbT,
        "em": Em,
        "eb": Eb,
        "fc1w": np.ascontiguousarray(np.asarray(fc1_w, np.float32).T),
        "fc1b": np.asarray(fc1_b, np.float32).reshape(C, 1),
        "fc2w": np.ascontiguousarray(np.asarray(fc2_w, np.float32).T),
        "fc2b": np.asarray(fc2_b, np.float32).reshape(C, 1),
        "whh": np.ascontiguousarray(whh_eff.T),                      # [64, 256]
        "bh0": np.ascontiguousarray(b_all[:128].reshape(128, 1)),
        "bh1": np.ascontiguousarray(b_all[128:].reshape(128, 1)),
        "fcw": np.ascontiguousarray(fcw_eff.T),                      # [64, 3]
        "fcb": np.asarray(fc_b, np.float32).reshape(3, 1),
    }


def _shard_x(x):
    """x [B, T, C, 3,3,3] -> per-core [NCH, 128, NTOK] fp32, tok = t*16 + b."""
    x = np.asarray(x, np.float32).reshape(B, T, KIN)
    shards = []
    for c in range(NCORES):
        xc = x[c * BL:(c + 1) * BL]                      # [16, T, 1728]
        xt = np.ascontiguousarray(xc.transpose(2, 1, 0)).reshape(KIN, NTOK)
        xp = np.zeros((KPAD, NTOK), np.float32)
        xp[:KIN] = xt
        shards.append(np.ascontiguousarray(xp.reshape(NCH, 128, NTOK)))
    return shards


# ---------------------------------------------------------------- device build
def _build():
    nc = bacc.Bacc("TRN2", target_bir_lowering=False)
    d_x = nc.dram_tensor("xT", [NCH, 128, NTOK], FP32, kind="ExternalInput")
    d_wbig = nc.dram_tensor("wbigT", [NCH, 128, 256], FP32, kind="ExternalInput")
    d_em = nc.dram_tensor("em", [NCH, 128, C], FP32, kind="ExternalInput")
    d_eb = nc.dram_tensor("eb", [NCH, C, 128], FP32, kind="ExternalInput")
    d_fc1w = nc.dram_tensor("fc1w", [C, C], FP32, kind="ExternalInput")
    d_fc1b = nc.dram_tensor("fc1b", [C, 1], FP32, kind="ExternalInput")
    d_fc2w = nc.dram_tensor("fc2w", [C, C], FP32, kind="ExternalInput")
    d_fc2b = nc.dram_tensor("fc2b", [C, 1], FP32, kind="ExternalInput")
    d_whh = nc.dram_tensor("whh", [HID, 256], FP32, kind="ExternalInput")
    d_bh0 = nc.dram_tensor("bh0", [128, 1], FP32, kind="ExternalInput")
    d_bh1 = nc.dram_tensor("bh1", [128, 1], FP32, kind="ExternalInput")
    d_fcw = nc.dram_tensor("fcw", [HID, 3], FP32, kind="ExternalInput")
    d_fcb = nc.dram_tensor("fcb", [3, 1], FP32, kind="ExternalInput")
    d_out = nc.dram_tensor("out", [3, BL], FP32, kind="ExternalOutput")

    TANH = mybir.ActivationFunctionType.Tanh
    EXP = mybir.ActivationFunctionType.Exp
    IDENT = mybir.ActivationFunctionType.Identity

    with tile.TileContext(nc) as tc, ExitStack() as ctx:
        consts = ctx.enter_context(tc.tile_pool(name="consts", bufs=1))
        xpool = ctx.enter_context(tc.tile_pool(name="x", bufs=2))
        xapool = ctx.enter_context(tc.tile_pool(name="xa", bufs=2))
        gxpool = ctx.enter_context(tc.tile_pool(name="gx", bufs=NBLK))
        small = ctx.enter_context(tc.tile_pool(name="small", bufs=3))
        state = ctx.enter_context(tc.tile_pool(name="state", bufs=3))
        ps_stat = ctx.enter_context(tc.tile_pool(name="ps_stat", bufs=1, space="PSUM"))
        ps_ab = ctx.enter_context(tc.tile_pool(name="ps_ab", bufs=2, space="PSUM"))
        ps_g = ctx.enter_context(tc.tile_pool(name="ps_g", bufs=2, space="PSUM"))
        ps_gate = ctx.enter_context(tc.tile_pool(name="ps_gate", bufs=3, space="PSUM"))

        # ---- constants
        wbig = consts.tile([128, NCH, 256], FP32)
        nc.sync.dma_start(wbig[:], d_wbig.ap().rearrange("c p f -> p c f"))
        em = consts.tile([128, NCH, C], FP32)
        nc.sync.dma_start(em[:], d_em.ap().rearrange("c p f -> p c f"))
        eb = consts.tile([C, NCH, 128], FP32)
        nc.sync.dma_start(eb[:], d_eb.ap().rearrange("c p f -> p c f"))
        fc1w = consts.tile([C, C], FP32); nc.sync.dma_start(fc1w[:], d_fc1w.ap())
        fc1b = consts.tile([C, 1], FP32); nc.sync.dma_start(fc1b[:], d_fc1b.ap())
        fc2w = consts.tile([C, C], FP32); nc.sync.dma_start(fc2w[:], d_fc2w.ap())
        fc2b = consts.tile([C, 1], FP32); nc.sync.dma_start(fc2b[:], d_fc2b.ap())
        whh = consts.tile([HID, 256], FP32); nc.sync.dma_start(whh[:], d_whh.ap())
        bh0 = consts.tile([128, 1], FP32); nc.sync.dma_start(bh0[:], d_bh0.ap())
        bh1 = consts.tile([128, 1], FP32); nc.sync.dma_start(bh1[:], d_bh1.ap())
        fcw = consts.tile([HID, 3], FP32); nc.sync.dma_start(fcw[:], d_fcw.ap())
        fcb = consts.tile([3, 1], FP32); nc.sync.dma_start(fcb[:], d_fcb.ap())
        id128 = consts.tile([128, 128], FP32); make_identity(nc, id128[:])
        id64 = consts.tile([64, 64], FP32); make_identity(nc, id64[:])
        hsT = consts.tile([HID, BL, T], FP32)      # H history, [64, b, t]

        gx_blocks = []

        # ================= feedforward over all tokens, block by block ======
        for blk in range(NBLK):
            n0 = blk * BLKTOK
            x_blk = xpool.tile([128, NCH, BLKTOK], FP32, tag="x")
            for ch in range(NCH):
                nc.sync.dma_start(x_blk[:, ch, :], d_x.ap()[ch, :, n0:n0 + BLKTOK])

            # channel means: accumulate Em.T @ x over chunks -> [C, 512]
            xavg_ps = ps_stat.tile([C, BLKTOK], FP32, tag="stat")
            for ch in range(NCH):
                nc.tensor.matmul(xavg_ps[:], em[:, ch, :], x_blk[:, ch, :],
                                 start=(ch == 0), stop=(ch == NCH - 1))
            xavg = small.tile([C, BLKTOK], FP32, tag="xavg")
            nc.scalar.copy(xavg[:], xavg_ps[:])

            # attention MLP (all [C, 512], c on partitions)
            a_ps = ps_stat.tile([C, BLKTOK], FP32, tag="stat")
            nc.tensor.matmul(a_ps[:], fc1w[:], xavg[:], start=True, stop=True)
            a_sb = small.tile([C, BLKTOK], FP32, tag="a")
            nc.scalar.activation(a_sb[:], a_ps[:], TANH, bias=fc1b[:])
            l_ps = ps_stat.tile([C, BLKTOK], FP32, tag="stat")
            nc.tensor.matmul(l_ps[:], fc2w[:], a_sb[:], start=True, stop=True)
            e_sb = small.tile([C, BLKTOK], FP32, tag="e")
            nc.scalar.activation(e_sb[:], l_ps[:], EXP, bias=fc2b[:])

            # softmax normalization via transpose: sums over c per token
            et_ps = ps_stat.tile([128, NG, C], FP32, tag="stat")
            for q in range(NG):
                nc.tensor.matmul(et_ps[:, q, :], e_sb[:, q * 128:(q + 1) * 128],
                                 id64[:], is_transpose=True,
                                 start=(q == 0), stop=(q == NG - 1),
                                 skip_group_check=True)
            ssum = small.tile([128, NG], FP32, tag="ssum")
            nc.vector.tensor_reduce(out=ssum[:], in_=et_ps[:],
                                    op=AL.add, axis=mybir.AxisListType.X)
            sinv = small.tile([128, NG], FP32, tag="sinv")
            nc.vector.reciprocal(sinv[:], ssum[:])
            att_t = small.tile([128, NG, C], FP32, tag="att_t")
            for q in range(NG):
                nc.vector.tensor_scalar_mul(att_t[:, q, :], et_ps[:, q, :],
                                            sinv[:, q:q + 1])
            attT_ps = ps_stat.tile([C, BLKTOK], FP32, tag="stat")
            for q in range(NG):
                nc.tensor.matmul(attT_ps[:, q * 128:(q + 1) * 128], att_t[:, q, :],
                                 id128[:], is_transpose=True,
                                 start=(q == 0), stop=(q == NG - 1),
                                 skip_group_check=True)
            attT = small.tile([C, BLKTOK], FP32, tag="attT")
            nc.scalar.copy(attT[:], attT_ps[:])

            # x * att (broadcast over the 27 spatial positions via Eb matmul)
            xa_blk = xapool.tile([128, NCH, BLKTOK], FP32, tag="xa")
            for ch in range(NCH):
                ab_ps = ps_ab.tile([128, BLKTOK], FP32, tag="ab")
                nc.tensor.matmul(ab_ps[:], eb[:, ch, :], attT[:],
                                 start=True, stop=True)
                nc.vector.tensor_mul(xa_blk[:, ch, :], x_blk[:, ch, :], ab_ps[:])

            # big matmul: gates_x = W_big @ x_a  (+bias on copy-out)
            gx_blk = gxpool.tile([128, TBLK, 2, BL], FP32, tag="gx")
            for half in range(2):
                g_ps = ps_g.tile([128, BLKTOK], FP32, tag="g")
                for ch in range(NCH):
                    nc.tensor.matmul(
                        g_ps[:], wbig[:, ch, half * 128:(half + 1) * 128],
                        xa_blk[:, ch, :],
                        start=(ch == 0), stop=(ch == NCH - 1))
                nc.scalar.activation(
                    gx_blk[:, :, half, :],
                    g_ps[:].rearrange("p (t b) -> p t b", b=BL),
                    IDENT, bias=(bh0[:] if half == 0 else bh1[:]))
            gx_blocks.append(gx_blk)

        # ================= LSTM scan =======================================
        S_prev = state.tile([HID, BL], FP32, tag="S")
        nc.vector.memset(S_prev[:], 0.0)
        H_prev = None
        for t in range(T):
            blk, ti = t // TBLK, t % TBLK
            gx_blk = gx_blocks[blk]
            g2 = ps_gate.tile([128, 2 * BL], FP32, tag="gate")
            nc.tensor.matmul(g2[:], id128[:], gx_blk[:, ti, :, :],
                             start=True, stop=True)
            if H_prev is not None:
                nc.tensor.matmul(g2[:, 0:BL], whh[:, 0:128], H_prev,
                                 start=False, stop=False, skip_group_check=True)
                nc.tensor.matmul(g2[:, BL:2 * BL], whh[:, 128:256], H_prev,
                                 start=False, stop=False, skip_group_check=True)
            # tau = tanh(gates/2); hi half relocated to partitions 0:64
            ta = small.tile([HID, 2 * BL], FP32, tag="ta")   # i | g
            tb = small.tile([HID, 2 * BL], FP32, tag="tb")   # f | o
            nc.scalar.activation(ta[:], g2[0:64, :], TANH, scale=0.5)
            nc.scalar.activation(tb[:], g2[64:128, :], TANH, scale=0.5)
            u1 = small.tile([HID, BL], FP32, tag="u1")
            nc.vector.scalar_tensor_tensor(u1[:], tb[:, 0:BL], 1.0, S_prev[:],
                                           AL.add, AL.mult)
            u2 = small.tile([HID, BL], FP32, tag="u2")
            nc.vector.scalar_tensor_tensor(u2[:], ta[:, 0:BL], 1.0, ta[:, BL:2 * BL],
                                           AL.add, AL.mult)
            S = state.tile([HID, BL], FP32, tag="S")
            nc.vector.scalar_tensor_tensor(S[:], u1[:], 0.5, u2[:],
                                           AL.mult, AL.add)
            tc_t = small.tile([HID, BL], FP32, tag="tc")
            nc.scalar.activation(tc_t[:], S[:], TANH, scale=0.5)
            H = hsT[:, :, t]                                  # [64, b] stride T
            nc.vector.scalar_tensor_tensor(H, tb[:, BL:2 * BL], 1.0, tc_t[:],
                                           AL.add, AL.mult)
            S_prev, H_prev = S, H

        # ================= head ============================================
        hsum = small.tile([HID, BL], FP32, tag="hsum")
        nc.vector.tensor_reduce(out=hsum[:], in_=hsT[:], op=AL.add,
                                axis=mybir.AxisListType.X)
        o_ps = ps_stat.tile([3, BL], FP32, tag="stat")
        nc.tensor.matmul(o_ps[:], fcw[:], hsum[:], start=True, stop=True)
        o_sb = small.tile([3, BL], FP32, tag="o")
        nc.scalar.activation(o_sb[:], o_ps[:], IDENT, bias=fcb[:])
        nc.sync.dma_start(d_out.ap(), o_sb[:])

    nc.compile()
    return nc


def _get_nc():
    if "nc" not in _CACHE:
        _CACHE["nc"] = _build()
    return _CACHE["nc"]


# ---------------------------------------------------------------- entry point
def kernel(x, fc1_w, fc1_b, fc2_w, fc2_b, conv_w, conv_b,
           w_ih, w_hh, b_ih, b_hh, fc_w, fc_b, _trace=False, _trace_kwargs=None):
    consts = _fold_weights(fc1_w, fc1_b, fc2_w, fc2_b, conv_w, conv_b,
                           w_ih, w_hh, b_ih, b_hh, fc_w, fc_b)
    shards = _shard_x(x)
    in_maps = [dict(consts, xT=shards[c]) for c in range(NCORES)]
    nc = _get_nc()
    res = run_bass_kernel_spmd(nc, in_maps, list(range(NCORES)),
                               trace=_trace, **(_trace_kwargs or {}))
    out = np.concatenate([res.results[c]["out"].T for c in range(NCORES)], axis=0)
    if _trace:
        return out.astype(np.float32), res
    return out.astype(np.float32)
